# revision 11
# baseline (speedup 1.0000x reference)
"""Trainium2 Bass kernel for nn_CausalTemporalAttention.

Reference semantics (B == L == H == 8 required by the module's broadcast quirks):
  qkv = x @ w_qkv.T + b_qkv ; split q,k,v -> [B,L,H,S,d]
  scores[b,l,h,s,t] = q.k/sqrt(d) ; masked to -1e9 where h > l
  z = scores * decay_params[b,l,h] ; attn = softmax over l (the layer axis)
  out[b,l,h,s,:] = attn @ v ; swap (l,h) ; row-major reshape to [B*H, S, E]
  y = out @ w_out.T + b_out ; reshape [B,L,S,E]

Sharding: data-parallel over batch B across 8 cores (core i handles b=i).
All matmuls run in bf16 with fp32 PSUM accumulation; softmax in fp32->bf16.

Structural shortcuts (exact, not approximations):
  * head 7: softmax over a single unmasked layer -> attn == 1, so its
    output rows are the column-sum of v[7,7] broadcast over s. No q/k
    projection, no scores, no softmax for head 7.
  * head 6: 2-way softmax -> sigmoid of the scaled score difference.
    No reciprocal on the tail critical path.
  * out-projection operates on PACKED rows (the h>l zero rows are never
    computed, written, or projected); the host re-inserts the zeros.
  * tokens are permuted within each layer on the host so the attn@v
    output lands directly in the out-projection's scrambled layout.
"""

import os
import sys

import numpy as np
import ml_dtypes

if "/opt/trn_rl_repo" not in sys.path:
    sys.path.insert(0, "/opt/trn_rl_repo")

B, L, S, E = 8, 8, 256, 1024
H, d = 8, E // 8
T = L * S            # 2048 tokens per batch element
NE = E // 128        # 8 e-chunks
F = 3 * E

# (l, h) pairs with h <= l, l-major so fixed-l head blocks are contiguous
PAIRS = [(l, h) for l in range(L) for h in range(l + 1)]
BLK = {p: i for i, p in enumerate(PAIRS)}  # 36 blocks

# packed out-projection rows: head h contributes rows s' in [32h, 256)
OFF = [0]
for h in range(H):
    OFF.append(OFF[-1] + (S - 32 * h))     # [0,256,480,672,832,960,1056,1120,1152]
R = OFF[-1]                                 # 1152 packed rows
NCHUNK = R // 128                           # 9 chunks of 128 rows
# chunks that become emittable once head h's attn@v rows are written
# (head-7 rows are prefilled in the front, so chunk 8 is ready after head 6)
READY_AFTER = {0: [0, 1], 1: [2], 2: [3, 4], 3: [5], 4: [6], 5: [7], 6: [8]}

_BUILD_CACHE = {}


def _build(with_bias):
    import concourse.bass as bass
    import concourse.tile as tile
    import concourse.mybir as mybir
    from concourse import bacc
    from contextlib import ExitStack

    dt = mybir.dt
    AF = mybir.ActivationFunctionType

    nc = bacc.Bacc("TRN2", target_bir_lowering=False, debug=False, num_devices=8)

    xT_d = nc.dram_tensor("xT", [E, T], dt.bfloat16, kind="ExternalInput").ap()
    # q/k weights pre-packed on host as [part, head, p, e*128+m] so each
    # (part, head) loads with ONE contiguous-per-partition DMA.
    wqkp_d = nc.dram_tensor("wqkp", [2, H, 128, E], dt.bfloat16, kind="ExternalInput").ap()
    wv_d = nc.dram_tensor("wvT", [E, E], dt.bfloat16, kind="ExternalInput").ap()
    wo_d = nc.dram_tensor("woutT", [E, E], dt.bfloat16, kind="ExternalInput").ap()
    dec_d = nc.dram_tensor("decay", [128, 2 * L * H], dt.float32, kind="ExternalInput").ap()
    if with_bias:
        bq_d = nc.dram_tensor("bqkv", [1, F], dt.bfloat16, kind="ExternalInput").ap()
    yp_d = nc.dram_tensor("yp", [R, E], dt.bfloat16, kind="ExternalOutput").ap()

    with ExitStack() as ctx:
        ctx.enter_context(
            nc.allow_low_precision(
                reason="bf16 softmax intermediates; end-to-end error ~5e-3 of scale"
            )
        )
        tc = ctx.enter_context(tile.TileContext(nc))

        consts = ctx.enter_context(tc.tile_pool(name="consts", bufs=1))
        # One tile + one DMA writer per chunk: a tile written by DMAs spread
        # over many HW queues gives its first consumer more sync waits than
        # the MM instruction can encode ("Too many sync wait commands"), and
        # fine-grained tiles let compute start as soon as its chunk lands.
        xT_sb = [consts.tile([128, T], dt.bfloat16, name=f"xT{e}") for e in range(NE)]
        wqv_sb = [consts.tile([128, E], dt.bfloat16, name=f"wqv{e}") for e in range(NE)]
        # q/k weights: one tile per (part, head); head 7 is never used.
        wqk_sb = {
            (part, h): consts.tile([128, E], dt.bfloat16, name=f"w{part}{h}")
            for part in ("q", "k")
            for h in range(H - 1)
        }
        wo_sb = [consts.tile([128, E], dt.bfloat16, name=f"wo{e}") for e in range(NE)]
        dec_sb = consts.tile([128, 2 * L * H], dt.float32)
        v_sb = consts.tile([128, len(PAIRS), 2, d], dt.bfloat16)
        # packed transposed out-proj input: gt[dd, j, row]
        gt_sb = consts.tile([128, NE, R], dt.bfloat16, name="gt_packed")
        ones_sb = consts.tile([128, 512], dt.bfloat16, name="ones")

        if with_bias:
            bq_sb = consts.tile([1, F], dt.bfloat16)
            nc.scalar.dma_start(out=bq_sb, in_=bq_d)

        nc.vector.memset(ones_sb, 1.0)
        nc.scalar.dma_start(out=dec_sb, in_=dec_d)

        def _wqk_dma(eng, part, pi, h):
            for half in range(2):
                eng.dma_start(
                    out=wqk_sb[(part, h)][:, half * 512:(half + 1) * 512],
                    in_=wqkp_d[pi, h, :, half * 512:(half + 1) * 512],
                )

        # DMA issue order == consumption order on BOTH queues, with the
        # front-critical bytes (head-0 weights, xT, w_v) split evenly across
        # the two DGE queues; late-needed weights load after them.
        #   SP:  head-0 q/k weights, xT quarters (e-chunks 0-4), head-1, w_out
        #   ACT: per quarter: w_v column group then xT e-chunks 5-7; heads 2-6
        for pi, part in ((0, "q"), (1, "k")):
            _wqk_dma(nc.sync, part, pi, 0)
        for q in range(4):
            for e in range(5):
                nc.sync.dma_start(
                    out=xT_sb[e][:, q * 512:(q + 1) * 512],
                    in_=xT_d[e * 128:(e + 1) * 128, q * 512:(q + 1) * 512],
                )
            for e in range(NE):
                nc.scalar.dma_start(
                    out=wqv_sb[e][:, q * 256:(q + 1) * 256],
                    in_=wv_d[e * 128:(e + 1) * 128, q * 256:(q + 1) * 256],
                )
            for e in range(5, NE):
                nc.scalar.dma_start(
                    out=xT_sb[e][:, q * 512:(q + 1) * 512],
                    in_=xT_d[e * 128:(e + 1) * 128, q * 512:(q + 1) * 512],
                )
        for pi, part in ((0, "q"), (1, "k")):
            _wqk_dma(nc.sync, part, pi, 1)
        for e in range(NE):
            for half in range(2):
                nc.sync.dma_start(
                    out=wo_sb[e][:, half * 512:(half + 1) * 512],
                    in_=wo_d[e * 128:(e + 1) * 128, half * 512:(half + 1) * 512],
                )
        for h in range(2, H - 1):
            for pi, part in ((0, "q"), (1, "k")):
                _wqk_dma(nc.scalar, part, pi, h)

        mm_ps = ctx.enter_context(tc.tile_pool(name="mm_ps", bufs=3, space="PSUM"))
        sc_ps = ctx.enter_context(tc.tile_pool(name="sc_ps", bufs=3, space="PSUM"))
        o2_ps = ctx.enter_context(tc.tile_pool(name="o2_ps", bufs=2, space="PSUM"))

        qk_pool = ctx.enter_context(tc.tile_pool(name="qk", bufs=2))
        exp_pool = ctx.enter_context(tc.tile_pool(name="expp", bufs=2))
        sm_pool = ctx.enter_context(tc.tile_pool(name="smp", bufs=2))
        at_pool = ctx.enter_context(tc.tile_pool(name="atp", bufs=3))
        out_pool = ctx.enter_context(tc.tile_pool(name="outp", bufs=3))

        def v_proj(tts):
            # v projection (natural [token, dd] layout): stationary xT tile,
            # moving w columns. Only heads h <= l are ever read.
            for tt in tts:
                l = tt // 2
                ncols = 128 * (l + 1)
                for g in range((ncols + 511) // 512):
                    n_g = min(512, ncols - 512 * g)
                    p_v = mm_ps.tile([128, n_g], dt.float32, tag="mm", name="p_v")
                    for e in range(NE):
                        nc.tensor.matmul(
                            p_v,
                            lhsT=xT_sb[e][:, tt * 128:(tt + 1) * 128],
                            rhs=wqv_sb[e][:, 512 * g: 512 * g + n_g],
                            start=(e == 0),
                            stop=(e == NE - 1) and not with_bias,
                        )
                    if with_bias:
                        nc.tensor.matmul(
                            p_v,
                            lhsT=ones_sb[0:1, :128],
                            rhs=bq_sb[:, 2 * E + 512 * g: 2 * E + 512 * g + n_g],
                            start=False,
                            stop=True,
                        )
                    nh = n_g // 128
                    blk0 = BLK[(l, 4 * g)]
                    nc.vector.tensor_copy(
                        out=v_sb[:, blk0:blk0 + nh, tt % 2, :],
                        in_=p_v.rearrange("p (hh dd) -> p hh dd", hh=nh),
                    )

        # ---- per-head pipeline helpers
        def qk_pair(h, part, base, dst, l, nl):
            p_qk = mm_ps.tile([128, nl * S], dt.float32, tag="mm", name="p_qk")
            for e in range(NE):
                nc.tensor.matmul(
                    p_qk,
                    lhsT=wqk_sb[(part, h)][:, e * 128:(e + 1) * 128],
                    rhs=xT_sb[e][:, l * S:(l + nl) * S],
                    start=(e == 0),
                    stop=(e == NE - 1) and not with_bias,
                )
            if with_bias:
                nc.tensor.matmul(
                    p_qk,
                    lhsT=bq_sb[:, base + h * 128: base + (h + 1) * 128],
                    rhs=ones_sb[0:1, :nl * S],
                    start=False,
                    stop=True,
                )
            src = p_qk.rearrange("p (a b) -> p a b", a=nl)
            nc.scalar.copy(out=dst[:, l:l + nl, :], in_=src)

        def layer_pairs(h):
            out, l = [], h
            while l < L:
                nl = 2 if l + 1 < L else 1  # pair layers: N=512 moving dim
                out.append((l, nl))
                l += nl
            return out

        def emit_chunk(c):
            # out projection for packed rows [128c, 128c+128): stationary gt
            # rows, moving w_out columns.
            for ng in range(2):
                p_pr = mm_ps.tile([128, 512], dt.float32, tag="mm", name="p_pr")
                for j in range(NE):
                    nc.tensor.matmul(
                        p_pr,
                        lhsT=gt_sb[:, j, c * 128:(c + 1) * 128],
                        rhs=wo_sb[j][:, ng * 512:(ng + 1) * 512],
                        start=(j == 0),
                        stop=(j == NE - 1),
                    )
                o_sb = out_pool.tile([128, 512], dt.bfloat16, tag="o", name="o_sb")
                nc.scalar.copy(out=o_sb, in_=p_pr)
                nc.sync.dma_start(
                    out=yp_d[c * 128:(c + 1) * 128, ng * 512:(ng + 1) * 512],
                    in_=o_sb,
                )

        def gt_scatter(h, l, p_o2):
            # p_o2 free index is s_p = j*32 + si (host token permutation), so
            # this is a contiguous-by-32 strided copy into the packed layout.
            r0 = OFF[h] + (l - h) * 32
            nc.scalar.copy(
                out=gt_sb[:, :, r0:r0 + 32],
                in_=p_o2.rearrange("p (j si) -> p j si", j=NE),
            )

        # ---- HAM warm-up: dummy matmuls with no DMA deps fill the input
        # lead-in and open the PE clock gate before real work arrives.
        warm_w = consts.tile([128, 128], dt.bfloat16, name="warm_w")
        warm_x = consts.tile([128, 512], dt.bfloat16, name="warm_x")
        nc.vector.memset(warm_w, 0.0)
        nc.vector.memset(warm_x, 0.0)
        p_warm = mm_ps.tile([128, 512], dt.float32, tag="mm", name="p_warm")
        for i in range(20):
            nc.tensor.matmul(p_warm, lhsT=warm_w, rhs=warm_x,
                             start=(i == 0), stop=(i == 19))

        # ---- front: head-0 q/k pairs and v-groups interleaved by xT quarter,
        # matching DMA arrival order so the in-order PE never blocks on a
        # not-yet-loaded chunk.
        qT0 = qk_pool.tile([128, L, S], dt.bfloat16, tag="qT", name="qT_sb")
        kT0 = qk_pool.tile([128, L, S], dt.bfloat16, tag="kT", name="kT_sb")
        for p in range(4):
            qk_pair(0, "q", 0, qT0, 2 * p, 2)
            qk_pair(0, "k", E, kT0, 2 * p, 2)
            v_proj(range(4 * p, 4 * p + 4))

        # head 7: attn == 1, so out rows are the column-sum of v[7,7],
        # broadcast over s. ones-rhs matmul -> normal gt scatter.
        p_bc = o2_ps.tile([128, S], dt.float32, tag="o2", name="p_bc")
        for tc2 in range(2):
            nc.tensor.matmul(
                p_bc,
                lhsT=v_sb[:, BLK[(7, 7)], tc2, :],
                rhs=ones_sb[:, :S],
                start=(tc2 == 0),
                stop=(tc2 == 1),
            )
        gt_scatter(7, 7, p_bc)

        qk_tiles = (qT0, kT0)
        pending = []
        for h in range(H - 1):
            qT_sb, kT_sb = qk_tiles
            # next head's q/k projection: q layer-pairs are interleaved INTO
            # the scores loop so the ACT queue alternates EXP / qT-copy and
            # the PE has filler between score matmuls; k layer-pairs follow
            # the reciprocal issue.
            if h + 1 < H - 1:
                qT_n = qk_pool.tile([128, L, S], dt.bfloat16, tag="qT", name="qT_sb")
                kT_n = qk_pool.tile([128, L, S], dt.bfloat16, tag="kT", name="kT_sb")
                qk_tiles = (qT_n, kT_n)
                next_pairs = layer_pairs(h + 1)
            else:
                next_pairs = []
            if h < 6:
                # scores (transposed [t, s]) + exp with decay/sqrt(d) folded
                # into the activation scale; D accumulates the denominator.
                E_sb = exp_pool.tile([128, L, 2, S], dt.bfloat16, tag="E", name="E_sb")
                D_sb = sm_pool.tile([128, 2, S], dt.bfloat16, tag="D", name="D_sb")
                for i, l in enumerate(range(h, L)):
                    p_sc = sc_ps.tile([128, 2, S], dt.float32, tag="sc", name="p_sc")
                    for tc2 in range(2):
                        nc.tensor.matmul(
                            p_sc[:, tc2, :],
                            lhsT=kT_sb[:, l, tc2 * 128:(tc2 + 1) * 128],
                            rhs=qT_sb[:, l, :],
                            start=True,
                            stop=True,
                        )
                    idx = l * H + h
                    nc.scalar.activation(
                        out=E_sb[:, l, :, :],
                        in_=p_sc,
                        func=AF.Exp,
                        scale=dec_sb[:, idx:idx + 1],
                    )
                    if l == h:
                        nc.vector.tensor_copy(out=D_sb, in_=E_sb[:, l, :, :])
                    else:
                        nc.vector.tensor_add(D_sb, D_sb, E_sb[:, l, :, :])
                    if i < len(next_pairs):
                        ln, nln = next_pairs[i]
                        qk_pair(h + 1, "q", 0, qk_tiles[0], ln, nln)
                # reciprocal issued BEFORE the k projection so it runs on DVE
                # while the PE streams matmuls.
                U_sb = sm_pool.tile([128, 2, S], dt.bfloat16, tag="U", name="U_sb")
                nc.vector.reciprocal(out=U_sb, in_=D_sb)
                ats = None
            else:
                # head 6: softmax over {6,7} == sigmoid(z6 - z7)
                p_s6 = sc_ps.tile([128, 2, S], dt.float32, tag="sc", name="p_s6")
                p_s7 = sc_ps.tile([128, 2, S], dt.float32, tag="sc", name="p_s7")
                for l, p_s in ((6, p_s6), (7, p_s7)):
                    for tc2 in range(2):
                        nc.tensor.matmul(
                            p_s[:, tc2, :],
                            lhsT=kT_sb[:, l, tc2 * 128:(tc2 + 1) * 128],
                            rhs=qT_sb[:, l, :],
                            start=True,
                            stop=True,
                        )
                z6 = at_pool.tile([128, 2, S], dt.bfloat16, tag="at", name="z6")
                z7m = at_pool.tile([128, 2, S], dt.bfloat16, tag="at", name="z7m")
                i66, i76 = 6 * H + 6, 7 * H + 6
                nc.scalar.activation(out=z6, in_=p_s6, func=AF.Copy,
                                     scale=dec_sb[:, i66:i66 + 1])
                nc.scalar.activation(out=z7m, in_=p_s7, func=AF.Copy,
                                     scale=dec_sb[:, 64 + i76:64 + i76 + 1])
                d67 = sm_pool.tile([128, 2, S], dt.bfloat16, tag="U", name="d67")
                nc.vector.tensor_add(d67, z6, z7m)
                at6 = at_pool.tile([128, 2, S], dt.bfloat16, tag="at", name="at6")
                at7 = at_pool.tile([128, 2, S], dt.bfloat16, tag="at", name="at7")
                nc.scalar.activation(out=at6, in_=d67, func=AF.Sigmoid)
                nc.scalar.activation(out=at7, in_=d67, func=AF.Sigmoid, scale=-1.0)
                ats = {6: at6, 7: at7}

            # k projection + ready out-proj chunks fill the PE while this
            # head's softmax chain finishes on ACT/DVE
            for ln, nln in next_pairs:
                qk_pair(h + 1, "k", E, qk_tiles[1], ln, nln)
            for c in pending:
                emit_chunk(c)
            pending = READY_AFTER[h]

            # attn @ v (output transposed [dd, s_p]) scattered into the
            # packed out-projection input
            for l in range(h, L):
                if ats is None:
                    at_sb = at_pool.tile([128, 2, S], dt.bfloat16, tag="at", name="at_sb")
                    nc.vector.tensor_mul(at_sb, E_sb[:, l, :, :], U_sb)
                else:
                    at_sb = ats[l]
                p_o2 = o2_ps.tile([128, S], dt.float32, tag="o2", name="p_o2")
                for tc2 in range(2):
                    nc.tensor.matmul(
                        p_o2,
                        lhsT=v_sb[:, BLK[(l, h)], tc2, :],
                        rhs=at_sb[:, tc2, :],
                        start=(tc2 == 0),
                        stop=(tc2 == 1),
                    )
                gt_scatter(h, l, p_o2)

        for c in pending:
            emit_chunk(c)

    nc.compile()
    return nc


def _get_nc(with_bias):
    if with_bias not in _BUILD_CACHE:
        _BUILD_CACHE[with_bias] = _build(with_bias)
    return _BUILD_CACHE[with_bias]


# token permutation within each layer: column p holds original position
# s = (p % 32) * 8 + p // 32, i.e. s_p = (s % 8) * 32 + s // 8
_PERM = (np.arange(S) % 32) * 8 + np.arange(S) // 32


def _prepare_in_maps(x, w_qkv, b_qkv, w_out, b_out, decay_params):
    bf16 = ml_dtypes.bfloat16
    with_bias = bool(np.any(b_qkv != 0))

    wqk_bf = w_qkv[:2 * E].astype(bf16)                          # [2E, E]
    # [part, head, m, e, p] -> [part, head, p, e, m]: each (part, head) tile
    # is the stationary lhsT for all e-chunks, contiguous in DRAM.
    wqkp = np.ascontiguousarray(
        wqk_bf.reshape(2, H, d, NE, 128).transpose(0, 1, 4, 3, 2)
    ).reshape(2, H, 128, E)
    wvT = np.ascontiguousarray(w_qkv[2 * E:].astype(bf16).T)     # [E, E]
    woutT = np.ascontiguousarray(w_out.astype(bf16).T)           # [E, E]

    in_maps = []
    for b in range(B):
        xp = x[b].reshape(L, S, E)[:, _PERM, :].reshape(T, E)
        xT = np.ascontiguousarray(xp.astype(bf16).T)             # [E, T]
        dec = (decay_params[b, :L, :H] / np.float32(np.sqrt(d))).astype(np.float32)
        dec2 = np.concatenate([dec.reshape(L * H), -dec.reshape(L * H)])
        dec2 = np.ascontiguousarray(np.broadcast_to(dec2[None, :], (128, 2 * L * H)))
        m = {"xT": xT, "wqkp": wqkp, "wvT": wvT, "woutT": woutT, "decay": dec2}
        if with_bias:
            m["bqkv"] = np.ascontiguousarray(b_qkv.astype(bf16).reshape(1, F))
        in_maps.append(m)
    return with_bias, in_maps


def _run(x, w_qkv, b_qkv, w_out, b_out, decay_params, **spmd_kwargs):
    from concourse.bass_utils import run_bass_kernel_spmd

    with_bias, in_maps = _prepare_in_maps(x, w_qkv, b_qkv, w_out, b_out, decay_params)
    nc = _get_nc(with_bias)
    res = run_bass_kernel_spmd(nc, in_maps, core_ids=list(range(B)), **spmd_kwargs)
    yp = np.stack([r["yp"] for r in res.results], axis=0).astype(np.float32)  # [B,R,E]
    out = np.zeros((B, H, S, E), dtype=np.float32)
    for h in range(H):
        out[:, h, 32 * h:, :] = yp[:, OFF[h]:OFF[h + 1], :]
    if np.any(b_out != 0):
        out += b_out.astype(np.float32)
    return out, res


def kernel(x, w_qkv, b_qkv, w_out, b_out, decay_params):
    out, _ = _run(
        np.asarray(x), np.asarray(w_qkv), np.asarray(b_qkv),
        np.asarray(w_out), np.asarray(b_out), np.asarray(decay_params),
    )
    return out


# revision 13
# speedup vs baseline: 1.1117x; 1.1117x over previous
"""Trainium2 Bass kernel for nn_CausalTemporalAttention.

Reference semantics (B == L == H == 8 required by the module's broadcast quirks):
  qkv = x @ w_qkv.T + b_qkv ; split q,k,v -> [B,L,H,S,d]
  scores[b,l,h,s,t] = q.k/sqrt(d) ; masked to -1e9 where h > l
  z = scores * decay_params[b,l,h] ; attn = softmax over l (the layer axis)
  out[b,l,h,s,:] = attn @ v ; swap (l,h) ; row-major reshape to [B*H, S, E]
  y = out @ w_out.T + b_out ; reshape [B,L,S,E]

Sharding: data-parallel over batch B across 8 cores (core i handles b=i).
All matmuls run in bf16 with fp32 PSUM accumulation; softmax in fp32->bf16.

Structural shortcuts (exact, not approximations):
  * head 7: softmax over a single unmasked layer -> attn == 1, so its
    output rows are the column-sum of v[7,7] broadcast over s. No q/k
    projection, no scores, no softmax for head 7.
  * head 6: 2-way softmax -> sigmoid of the scaled score difference.
    No reciprocal on the tail critical path.
  * out-projection operates on PACKED rows (the h>l zero rows are never
    computed, written, or projected); the host re-inserts the zeros.
  * tokens are permuted within each layer on the host so the attn@v
    output lands directly in the out-projection's scrambled layout.
"""

import os
import sys

import numpy as np
import ml_dtypes

if "/opt/trn_rl_repo" not in sys.path:
    sys.path.insert(0, "/opt/trn_rl_repo")

B, L, S, E = 8, 8, 256, 1024
H, d = 8, E // 8
T = L * S            # 2048 tokens per batch element
NE = E // 128        # 8 e-chunks
F = 3 * E

# (l, h) pairs with h <= l, l-major so fixed-l head blocks are contiguous
PAIRS = [(l, h) for l in range(L) for h in range(l + 1)]
BLK = {p: i for i, p in enumerate(PAIRS)}  # 36 blocks

# packed out-projection rows: head h contributes rows s' in [32h, 256)
OFF = [0]
for h in range(H):
    OFF.append(OFF[-1] + (S - 32 * h))     # [0,256,480,672,832,960,1056,1120,1152]
R = OFF[-1]                                 # 1152 packed rows
NCHUNK = R // 128                           # 9 chunks of 128 rows
# chunks that become emittable once head h's attn@v rows are written
# (head-7 rows are prefilled in the front, so chunk 8 is ready after head 6)
READY_AFTER = {0: [0, 1], 1: [2], 2: [3, 4], 3: [5], 4: [6], 5: [7], 6: [8]}

_BUILD_CACHE = {}


def _build(with_bias):
    import concourse.bass as bass
    import concourse.tile as tile
    import concourse.mybir as mybir
    from concourse import bacc
    from contextlib import ExitStack

    dt = mybir.dt
    AF = mybir.ActivationFunctionType

    nc = bacc.Bacc("TRN2", target_bir_lowering=False, debug=False, num_devices=8)

    xT_d = nc.dram_tensor("xT", [E, T], dt.bfloat16, kind="ExternalInput").ap()
    # q/k weights pre-packed on host as [part, head, p, e*128+m] so each
    # (part, head) loads with ONE contiguous-per-partition DMA.
    wqkp_d = nc.dram_tensor("wqkp", [2, H, 128, E], dt.bfloat16, kind="ExternalInput").ap()
    wv_d = nc.dram_tensor("wvT", [E, E], dt.bfloat16, kind="ExternalInput").ap()
    wo_d = nc.dram_tensor("woutT", [E, E], dt.bfloat16, kind="ExternalInput").ap()
    dec_d = nc.dram_tensor("decay", [128, 2 * L * H], dt.float32, kind="ExternalInput").ap()
    if with_bias:
        bq_d = nc.dram_tensor("bqkv", [1, F], dt.bfloat16, kind="ExternalInput").ap()
    yp_d = nc.dram_tensor("yp", [R, E], dt.bfloat16, kind="ExternalOutput").ap()

    with ExitStack() as ctx:
        ctx.enter_context(
            nc.allow_low_precision(
                reason="bf16 softmax intermediates; end-to-end error ~5e-3 of scale"
            )
        )
        tc = ctx.enter_context(tile.TileContext(nc))

        consts = ctx.enter_context(tc.tile_pool(name="consts", bufs=1))
        # One tile + one DMA writer per chunk: a tile written by DMAs spread
        # over many HW queues gives its first consumer more sync waits than
        # the MM instruction can encode ("Too many sync wait commands"), and
        # fine-grained tiles let compute start as soon as its chunk lands.
        xT_sb = [consts.tile([128, T], dt.bfloat16, name=f"xT{e}") for e in range(NE)]
        wqv_sb = [consts.tile([128, E], dt.bfloat16, name=f"wqv{e}") for e in range(NE)]
        # q/k weights: one tile per (part, head); head 7 is never used.
        wqk_sb = {
            (part, h): consts.tile([128, E], dt.bfloat16, name=f"w{part}{h}")
            for part in ("q", "k")
            for h in range(H - 1)
        }
        wo_sb = [consts.tile([128, E], dt.bfloat16, name=f"wo{e}") for e in range(NE)]
        dec_sb = consts.tile([128, 2 * L * H], dt.float32)
        v_sb = consts.tile([128, len(PAIRS), 2, d], dt.bfloat16)
        # packed transposed out-proj input: gt[dd, j, row]
        gt_sb = consts.tile([128, NE, R], dt.bfloat16, name="gt_packed")
        ones_sb = consts.tile([128, 512], dt.bfloat16, name="ones")

        if with_bias:
            bq_sb = consts.tile([1, F], dt.bfloat16)
            nc.scalar.dma_start(out=bq_sb, in_=bq_d)

        nc.vector.memset(ones_sb, 1.0)
        nc.scalar.dma_start(out=dec_sb, in_=dec_d)

        def _wqk_dma(eng, part, pi, h):
            eng.dma_start(out=wqk_sb[(part, h)], in_=wqkp_d[pi, h])

        # Each dma_start costs ~600ns of DGE descriptor-generation time, so
        # the front uses ONE start per tile (DRAM rows are contiguous), in
        # consumption order; late-needed weights queue after them.
        #   SP:  head-0 q/k weights, xT e-chunks, head-1 weights, w_out
        #   ACT: w_v e-chunks, then heads 2..6 (head 7 weights never needed)
        for pi, part in ((0, "q"), (1, "k")):
            _wqk_dma(nc.sync, part, pi, 0)
        for e in range(NE):
            nc.sync.dma_start(
                out=xT_sb[e], in_=xT_d[e * 128:(e + 1) * 128, :]
            )
        for pi, part in ((0, "q"), (1, "k")):
            _wqk_dma(nc.sync, part, pi, 1)
        for e in range(NE):
            nc.sync.dma_start(
                out=wo_sb[e], in_=wo_d[e * 128:(e + 1) * 128, :]
            )
        for e in range(NE):
            nc.scalar.dma_start(
                out=wqv_sb[e], in_=wv_d[e * 128:(e + 1) * 128, :]
            )
        for h in range(2, H - 1):
            for pi, part in ((0, "q"), (1, "k")):
                _wqk_dma(nc.scalar, part, pi, h)

        mm_ps = ctx.enter_context(tc.tile_pool(name="mm_ps", bufs=3, space="PSUM"))
        sc_ps = ctx.enter_context(tc.tile_pool(name="sc_ps", bufs=3, space="PSUM"))
        o2_ps = ctx.enter_context(tc.tile_pool(name="o2_ps", bufs=2, space="PSUM"))

        qk_pool = ctx.enter_context(tc.tile_pool(name="qk", bufs=2))
        exp_pool = ctx.enter_context(tc.tile_pool(name="expp", bufs=2))
        sm_pool = ctx.enter_context(tc.tile_pool(name="smp", bufs=2))
        at_pool = ctx.enter_context(tc.tile_pool(name="atp", bufs=3))
        out_pool = ctx.enter_context(tc.tile_pool(name="outp", bufs=3))

        def v_proj(tts):
            # v projection (natural [token, dd] layout): stationary xT tile,
            # moving w columns. Only heads h <= l are ever read.
            for tt in tts:
                l = tt // 2
                ncols = 128 * (l + 1)
                for g in range((ncols + 511) // 512):
                    n_g = min(512, ncols - 512 * g)
                    p_v = mm_ps.tile([128, n_g], dt.float32, tag="mm", name="p_v")
                    for e in range(NE):
                        nc.tensor.matmul(
                            p_v,
                            lhsT=xT_sb[e][:, tt * 128:(tt + 1) * 128],
                            rhs=wqv_sb[e][:, 512 * g: 512 * g + n_g],
                            start=(e == 0),
                            stop=(e == NE - 1) and not with_bias,
                        )
                    if with_bias:
                        nc.tensor.matmul(
                            p_v,
                            lhsT=ones_sb[0:1, :128],
                            rhs=bq_sb[:, 2 * E + 512 * g: 2 * E + 512 * g + n_g],
                            start=False,
                            stop=True,
                        )
                    nh = n_g // 128
                    blk0 = BLK[(l, 4 * g)]
                    nc.vector.tensor_copy(
                        out=v_sb[:, blk0:blk0 + nh, tt % 2, :],
                        in_=p_v.rearrange("p (hh dd) -> p hh dd", hh=nh),
                    )

        # ---- per-head pipeline helpers
        def qk_pair(h, part, base, dst, l, nl):
            p_qk = mm_ps.tile([128, nl * S], dt.float32, tag="mm", name="p_qk")
            for e in range(NE):
                nc.tensor.matmul(
                    p_qk,
                    lhsT=wqk_sb[(part, h)][:, e * 128:(e + 1) * 128],
                    rhs=xT_sb[e][:, l * S:(l + nl) * S],
                    start=(e == 0),
                    stop=(e == NE - 1) and not with_bias,
                )
            if with_bias:
                nc.tensor.matmul(
                    p_qk,
                    lhsT=bq_sb[:, base + h * 128: base + (h + 1) * 128],
                    rhs=ones_sb[0:1, :nl * S],
                    start=False,
                    stop=True,
                )
            src = p_qk.rearrange("p (a b) -> p a b", a=nl)
            nc.scalar.copy(out=dst[:, l:l + nl, :], in_=src)

        def layer_pairs(h):
            out, l = [], h
            while l < L:
                nl = 2 if l + 1 < L else 1  # pair layers: N=512 moving dim
                out.append((l, nl))
                l += nl
            return out

        def emit_chunk(c):
            # out projection for packed rows [128c, 128c+128): stationary gt
            # rows, moving w_out columns.
            for ng in range(2):
                p_pr = mm_ps.tile([128, 512], dt.float32, tag="mm", name="p_pr")
                for j in range(NE):
                    nc.tensor.matmul(
                        p_pr,
                        lhsT=gt_sb[:, j, c * 128:(c + 1) * 128],
                        rhs=wo_sb[j][:, ng * 512:(ng + 1) * 512],
                        start=(j == 0),
                        stop=(j == NE - 1),
                    )
                o_sb = out_pool.tile([128, 512], dt.bfloat16, tag="o", name="o_sb")
                nc.scalar.copy(out=o_sb, in_=p_pr)
                nc.sync.dma_start(
                    out=yp_d[c * 128:(c + 1) * 128, ng * 512:(ng + 1) * 512],
                    in_=o_sb,
                )

        def gt_scatter(h, l, p_o2):
            # p_o2 free index is s_p = j*32 + si (host token permutation), so
            # this is a contiguous-by-32 strided copy into the packed layout.
            r0 = OFF[h] + (l - h) * 32
            nc.scalar.copy(
                out=gt_sb[:, :, r0:r0 + 32],
                in_=p_o2.rearrange("p (j si) -> p j si", j=NE),
            )

        # ---- front: head-0 q/k pairs and v-groups interleaved by xT quarter,
        # matching DMA arrival order so the in-order PE never blocks on a
        # not-yet-loaded chunk.
        qT0 = qk_pool.tile([128, L, S], dt.bfloat16, tag="qT", name="qT_sb")
        kT0 = qk_pool.tile([128, L, S], dt.bfloat16, tag="kT", name="kT_sb")
        for p in range(4):
            qk_pair(0, "q", 0, qT0, 2 * p, 2)
            qk_pair(0, "k", E, kT0, 2 * p, 2)
            v_proj(range(4 * p, 4 * p + 4))

        # head 7: attn == 1, so out rows are the column-sum of v[7,7],
        # broadcast over s. ones-rhs matmul -> normal gt scatter.
        p_bc = o2_ps.tile([128, S], dt.float32, tag="o2", name="p_bc")
        for tc2 in range(2):
            nc.tensor.matmul(
                p_bc,
                lhsT=v_sb[:, BLK[(7, 7)], tc2, :],
                rhs=ones_sb[:, :S],
                start=(tc2 == 0),
                stop=(tc2 == 1),
            )
        gt_scatter(7, 7, p_bc)

        qk_tiles = (qT0, kT0)
        pending = []
        for h in range(H - 1):
            qT_sb, kT_sb = qk_tiles
            # next head's q/k projection: q layer-pairs are interleaved INTO
            # the scores loop so the ACT queue alternates EXP / qT-copy and
            # the PE has filler between score matmuls; k layer-pairs follow
            # the reciprocal issue.
            if h + 1 < H - 1:
                qT_n = qk_pool.tile([128, L, S], dt.bfloat16, tag="qT", name="qT_sb")
                kT_n = qk_pool.tile([128, L, S], dt.bfloat16, tag="kT", name="kT_sb")
                qk_tiles = (qT_n, kT_n)
                next_pairs = layer_pairs(h + 1)
            else:
                next_pairs = []
            if h < 6:
                # scores (transposed [t, s]) + exp with decay/sqrt(d) folded
                # into the activation scale; D accumulates the denominator.
                E_sb = exp_pool.tile([128, L, 2, S], dt.bfloat16, tag="E", name="E_sb")
                D_sb = sm_pool.tile([128, 2, S], dt.bfloat16, tag="D", name="D_sb")
                for i, l in enumerate(range(h, L)):
                    p_sc = sc_ps.tile([128, 2, S], dt.float32, tag="sc", name="p_sc")
                    for tc2 in range(2):
                        nc.tensor.matmul(
                            p_sc[:, tc2, :],
                            lhsT=kT_sb[:, l, tc2 * 128:(tc2 + 1) * 128],
                            rhs=qT_sb[:, l, :],
                            start=True,
                            stop=True,
                        )
                    idx = l * H + h
                    nc.scalar.activation(
                        out=E_sb[:, l, :, :],
                        in_=p_sc,
                        func=AF.Exp,
                        scale=dec_sb[:, idx:idx + 1],
                    )
                    if l == h:
                        nc.vector.tensor_copy(out=D_sb, in_=E_sb[:, l, :, :])
                    else:
                        nc.vector.tensor_add(D_sb, D_sb, E_sb[:, l, :, :])
                    if i < len(next_pairs):
                        ln, nln = next_pairs[i]
                        qk_pair(h + 1, "q", 0, qk_tiles[0], ln, nln)
                # reciprocal issued BEFORE the k projection so it runs on DVE
                # while the PE streams matmuls.
                U_sb = sm_pool.tile([128, 2, S], dt.bfloat16, tag="U", name="U_sb")
                nc.vector.reciprocal(out=U_sb, in_=D_sb)
                ats = None
            else:
                # head 6: softmax over {6,7} == sigmoid(z6 - z7)
                p_s6 = sc_ps.tile([128, 2, S], dt.float32, tag="sc", name="p_s6")
                p_s7 = sc_ps.tile([128, 2, S], dt.float32, tag="sc", name="p_s7")
                for l, p_s in ((6, p_s6), (7, p_s7)):
                    for tc2 in range(2):
                        nc.tensor.matmul(
                            p_s[:, tc2, :],
                            lhsT=kT_sb[:, l, tc2 * 128:(tc2 + 1) * 128],
                            rhs=qT_sb[:, l, :],
                            start=True,
                            stop=True,
                        )
                z6 = at_pool.tile([128, 2, S], dt.bfloat16, tag="at", name="z6")
                z7m = at_pool.tile([128, 2, S], dt.bfloat16, tag="at", name="z7m")
                i66, i76 = 6 * H + 6, 7 * H + 6
                nc.scalar.activation(out=z6, in_=p_s6, func=AF.Copy,
                                     scale=dec_sb[:, i66:i66 + 1])
                nc.scalar.activation(out=z7m, in_=p_s7, func=AF.Copy,
                                     scale=dec_sb[:, 64 + i76:64 + i76 + 1])
                d67 = sm_pool.tile([128, 2, S], dt.bfloat16, tag="U", name="d67")
                nc.vector.tensor_add(d67, z6, z7m)
                at6 = at_pool.tile([128, 2, S], dt.bfloat16, tag="at", name="at6")
                at7 = at_pool.tile([128, 2, S], dt.bfloat16, tag="at", name="at7")
                nc.scalar.activation(out=at6, in_=d67, func=AF.Sigmoid)
                nc.scalar.activation(out=at7, in_=d67, func=AF.Sigmoid, scale=-1.0)
                ats = {6: at6, 7: at7}

            # k projection + ready out-proj chunks fill the PE while this
            # head's softmax chain finishes on ACT/DVE
            for ln, nln in next_pairs:
                qk_pair(h + 1, "k", E, qk_tiles[1], ln, nln)
            for c in pending:
                emit_chunk(c)
            pending = READY_AFTER[h]

            # attn @ v (output transposed [dd, s_p]) scattered into the
            # packed out-projection input
            for l in range(h, L):
                if ats is None:
                    at_sb = at_pool.tile([128, 2, S], dt.bfloat16, tag="at", name="at_sb")
                    nc.vector.tensor_mul(at_sb, E_sb[:, l, :, :], U_sb)
                else:
                    at_sb = ats[l]
                p_o2 = o2_ps.tile([128, S], dt.float32, tag="o2", name="p_o2")
                for tc2 in range(2):
                    nc.tensor.matmul(
                        p_o2,
                        lhsT=v_sb[:, BLK[(l, h)], tc2, :],
                        rhs=at_sb[:, tc2, :],
                        start=(tc2 == 0),
                        stop=(tc2 == 1),
                    )
                gt_scatter(h, l, p_o2)

        for c in pending:
            emit_chunk(c)

    nc.compile()
    return nc


def _get_nc(with_bias):
    if with_bias not in _BUILD_CACHE:
        _BUILD_CACHE[with_bias] = _build(with_bias)
    return _BUILD_CACHE[with_bias]


# token permutation within each layer: column p holds original position
# s = (p % 32) * 8 + p // 32, i.e. s_p = (s % 8) * 32 + s // 8
_PERM = (np.arange(S) % 32) * 8 + np.arange(S) // 32


def _prepare_in_maps(x, w_qkv, b_qkv, w_out, b_out, decay_params):
    bf16 = ml_dtypes.bfloat16
    with_bias = bool(np.any(b_qkv != 0))

    wqk_bf = w_qkv[:2 * E].astype(bf16)                          # [2E, E]
    # [part, head, m, e, p] -> [part, head, p, e, m]: each (part, head) tile
    # is the stationary lhsT for all e-chunks, contiguous in DRAM.
    wqkp = np.ascontiguousarray(
        wqk_bf.reshape(2, H, d, NE, 128).transpose(0, 1, 4, 3, 2)
    ).reshape(2, H, 128, E)
    wvT = np.ascontiguousarray(w_qkv[2 * E:].astype(bf16).T)     # [E, E]
    woutT = np.ascontiguousarray(w_out.astype(bf16).T)           # [E, E]

    in_maps = []
    for b in range(B):
        xp = x[b].reshape(L, S, E)[:, _PERM, :].reshape(T, E)
        xT = np.ascontiguousarray(xp.astype(bf16).T)             # [E, T]
        dec = (decay_params[b, :L, :H] / np.float32(np.sqrt(d))).astype(np.float32)
        dec2 = np.concatenate([dec.reshape(L * H), -dec.reshape(L * H)])
        dec2 = np.ascontiguousarray(np.broadcast_to(dec2[None, :], (128, 2 * L * H)))
        m = {"xT": xT, "wqkp": wqkp, "wvT": wvT, "woutT": woutT, "decay": dec2}
        if with_bias:
            m["bqkv"] = np.ascontiguousarray(b_qkv.astype(bf16).reshape(1, F))
        in_maps.append(m)
    return with_bias, in_maps


def _run(x, w_qkv, b_qkv, w_out, b_out, decay_params, **spmd_kwargs):
    from concourse.bass_utils import run_bass_kernel_spmd

    with_bias, in_maps = _prepare_in_maps(x, w_qkv, b_qkv, w_out, b_out, decay_params)
    nc = _get_nc(with_bias)
    res = run_bass_kernel_spmd(nc, in_maps, core_ids=list(range(B)), **spmd_kwargs)
    yp = np.stack([r["yp"] for r in res.results], axis=0).astype(np.float32)  # [B,R,E]
    out = np.zeros((B, H, S, E), dtype=np.float32)
    for h in range(H):
        out[:, h, 32 * h:, :] = yp[:, OFF[h]:OFF[h + 1], :]
    if np.any(b_out != 0):
        out += b_out.astype(np.float32)
    return out, res


def kernel(x, w_qkv, b_qkv, w_out, b_out, decay_params):
    out, _ = _run(
        np.asarray(x), np.asarray(w_qkv), np.asarray(b_qkv),
        np.asarray(w_out), np.asarray(b_out), np.asarray(decay_params),
    )
    return out
